# revision 1
# baseline (speedup 1.0000x reference)
"""NeRF MLP forward pass on 8 Trainium2 NeuronCores (Bass/Tile).

Strategy: pure data parallel over rays. Each core processes 512 rays x 64
samples = 32768 points through the full MLP. Activations live transposed in
SBUF as [hidden, n_points] so every linear layer is a chain of
128x128 (stationary weight) x [128, 512] (moving activations) matmuls in
float32r (full-rate fp32 with 11-bit mantissa). Harmonic embeddings are
computed on-chip with Cody-Waite range reduction + the ScalarE Sin LUT.
"""

import sys

if '/opt/trn_rl_repo' not in sys.path:
    sys.path.insert(0, '/opt/trn_rl_repo')

import numpy as np

import concourse.bacc as bacc
import concourse.mybir as mybir
import concourse.tile as tile
from concourse.bass_utils import run_bass_kernel_spmd

F32 = mybir.dt.float32
F32R = mybir.dt.float32r
AF = mybir.ActivationFunctionType
ALU = mybir.AluOpType

N_CORES = 8
N_RAYS, S = 4096, 64
R_CORE = N_RAYS // N_CORES            # 512 rays per core
NPTS = R_CORE * S                     # 32768 points per core
# Points are ordered SAMPLE-major per core: point index = s * R_CORE + r.
# A 512-point sub-tile is then exactly one sample index across all rays,
# and the per-ray direction embedding broadcast is a contiguous block
# repeat.
F = 512                               # points per matmul (one PSUM bank)
FSUP = 2048                           # points per super-tile
NSUB = FSUP // F                      # 4
NSUP = NPTS // FSUP                   # 16
S_SUP = FSUP // R_CORE                # 4 samples per super-tile

H = 256
EMB_X = 63
EMB_D = 27

PI = float(np.pi)
INV2PI = float(1.0 / (2.0 * np.pi))
MAGIC = float(1.5 * 2 ** 23)
# Cody-Waite split of 2*pi: c1 exact in 9 mantissa bits, c2 in ~12, c3 rest.
_t = 2.0 * np.pi - 6.28125
_c2u = np.float32(_t).view(np.uint32) & np.uint32(0xFFFFF000)
CW1 = 6.28125
CW2 = float(_c2u.view(np.float32))
CW3 = float(np.float32(_t - float(_c2u.view(np.float32))))

# (name, kparts, out_chunks) ; kparts entries: (src, chunk_idx, ksize)
_XYZ_LAYERS = []
for li in range(8):
    if li == 0:
        kparts = [("E", 0, EMB_X)]
    elif li == 4:
        kparts = [("x", 0, 128), ("x", 1, 128), ("E", 0, EMB_X)]
    else:
        kparts = [("x", 0, 128), ("x", 1, 128)]
    _XYZ_LAYERS.append(kparts)

_cache = {}


def _build(nsup_exec=NSUP):
    """Build the bass program. nsup_exec > NSUP repeats super-tiles
    (st = i % NSUP) — used only for slope-based timing benchmarks."""
    key = ("nc", nsup_exec)
    if key in _cache:
        return _cache[key]

    nc = bacc.Bacc("TRN2", target_bir_lowering=False, debug=False,
                   num_devices=N_CORES)

    pts = nc.dram_tensor("pts", [3, NPTS], F32, kind="ExternalInput")
    dirs = nc.dram_tensor("dirs", [3, R_CORE], F32, kind="ExternalInput")
    w0 = nc.dram_tensor("w0", [EMB_X, 256], F32, kind="ExternalInput")
    wmid = {i: nc.dram_tensor(f"wmid{i}", [128, 512], F32, kind="ExternalInput")
            for i in range(1, 8)}
    w4e = nc.dram_tensor("w4e", [EMB_X, 256], F32, kind="ExternalInput")
    wfeat = nc.dram_tensor("wfeat", [128, 512], F32, kind="ExternalInput")
    wden = nc.dram_tensor("wden", [128, 2], F32, kind="ExternalInput")
    wd0 = nc.dram_tensor("wd0", [128, 256], F32, kind="ExternalInput")
    wd0e = nc.dram_tensor("wd0e", [EMB_D, 128], F32, kind="ExternalInput")
    wrgb = nc.dram_tensor("wrgb", [128, 3], F32, kind="ExternalInput")
    biases = nc.dram_tensor("biases", [128, 21], F32, kind="ExternalInput")
    consts = nc.dram_tensor("consts", [128, 4], F32, kind="ExternalInput")
    out = nc.dram_tensor("out", [4, NPTS], F32, kind="ExternalOutput")

    with tile.TileContext(nc) as tc:
        with (
            tc.tile_pool(name="wpool", bufs=1) as wpool,
            tc.tile_pool(name="epool", bufs=3) as epool,
            tc.tile_pool(name="spool", bufs=2) as spool,
            tc.tile_pool(name="apool", bufs=1) as apool,
            tc.tile_pool(name="opool", bufs=2) as opool,
            tc.tile_pool(name="psum", bufs=8, space="PSUM") as psum,
        ):
            # ---- load weights / constants (once) ----
            w0_t = wpool.tile([EMB_X, 256], F32R)
            nc.sync.dma_start(w0_t[:], w0[:].bitcast(F32R))
            wmid_t = {}
            for i in range(1, 8):
                wt = wpool.tile([128, 512], F32R, name=f"wmid{i}_t")
                nc.sync.dma_start(wt[:], wmid[i][:].bitcast(F32R))
                wmid_t[i] = wt
            w4e_t = wpool.tile([EMB_X, 256], F32R)
            nc.sync.dma_start(w4e_t[:], w4e[:].bitcast(F32R))
            wfeat_t = wpool.tile([128, 512], F32R)
            nc.sync.dma_start(wfeat_t[:], wfeat[:].bitcast(F32R))
            wden_t = wpool.tile([128, 2], F32R)
            nc.sync.dma_start(wden_t[:], wden[:].bitcast(F32R))
            wd0_t = wpool.tile([128, 256], F32R)
            nc.sync.dma_start(wd0_t[:], wd0[:].bitcast(F32R))
            wd0e_t = wpool.tile([EMB_D, 128], F32R)
            nc.sync.dma_start(wd0e_t[:], wd0e[:].bitcast(F32R))
            wrgb_t = wpool.tile([128, 3], F32R)
            nc.sync.dma_start(wrgb_t[:], wrgb[:].bitcast(F32R))
            b_t = wpool.tile([128, 21], F32)
            nc.sync.dma_start(b_t[:], biases[:])
            c_t = wpool.tile([128, 4], F32)
            nc.sync.dma_start(c_t[:], consts[:])
            zeros_t = wpool.tile([128, 1], F32)
            nc.any.memset(zeros_t[:], 0.0)

            def sincos(dst, scratch_p, scratch_k, freqshift, nrows):
                """dst[0:2*nrows] = [sin(a), cos(a)] with a = raw args
                replicated in both halves of scratch_p. freqshift is a
                [2*nrows, 2] fp32 AP: col0 per-row freq scale, col1 per-row
                shift (pi/2 for the cos half). Scratch is destroyed."""
                nc.vector.tensor_scalar(scratch_p[:], scratch_p[:],
                                        freqshift[:, 0:1], freqshift[:, 1:2],
                                        op0=ALU.mult, op1=ALU.add)
                nc.vector.tensor_scalar(scratch_k[:], scratch_p[:], INV2PI,
                                        MAGIC, op0=ALU.mult, op1=ALU.add)
                nc.vector.tensor_scalar(scratch_k[:], scratch_k[:], MAGIC,
                                        None, op0=ALU.subtract)
                nc.vector.cody_waite_cascade(scratch_p[:], scratch_p[:],
                                             scratch_k[:], CW1, CW2, CW3)
                nc.scalar.activation(dst[0:2 * nrows, :], scratch_p[:],
                                     AF.Sin, bias=zeros_t[0:2 * nrows, 0:1])

            # direction embedding tile (computed after emb_first below so the
            # xyz chain of super-tile 0 heads the in-order DVE/ACT queues)
            embd_rays = wpool.tile([EMB_D, R_CORE], F32R)
            pd = wpool.tile([24, R_CORE], F32)
            kd = wpool.tile([24, R_CORE], F32)

            # ---- per super-tile pipeline ----
            ev_counter = [0]

            def evict(psum_ap, out_ap, bias_ap):
                """relu(psum + bias) -> fp32r SBUF, alternating ACT/DVE."""
                if ev_counter[0] % 2 == 0:
                    nc.scalar.activation(out_ap, psum_ap, AF.Relu,
                                         bias=bias_ap)
                else:
                    nc.vector.tensor_scalar(out_ap, psum_ap, bias_ap, 0.0,
                                            op0=ALU.add, op1=ALU.max)
                ev_counter[0] += 1

            def emb_stages(st):
                """Generator emitting the harmonic-embedding pipeline for
                super-tile st one stage per next() call, so the serial DVE
                chain spreads through the in-order engine queues instead of
                landing as one burst. Final yield returns (E, embd)."""
                sl = slice(st * FSUP, (st + 1) * FSUP)
                P = spool.tile([60, FSUP], F32, name="P")
                K = spool.tile([60, FSUP], F32, name="K")
                for half in range(2):
                    for c in range(3):
                        nc.sync.dma_start(
                            P[half * 30 + c * 10:half * 30 + (c + 1) * 10, :],
                            pts[c:c + 1, sl].partition_broadcast(10))
                nc.vector.tensor_scalar(P[:], P[:], c_t[0:60, 0:1],
                                        c_t[0:60, 1:2],
                                        op0=ALU.mult, op1=ALU.add)
                yield None
                nc.vector.tensor_scalar(K[:], P[:], INV2PI, MAGIC,
                                        op0=ALU.mult, op1=ALU.add)
                yield None
                nc.vector.tensor_scalar(K[:], K[:], MAGIC, None,
                                        op0=ALU.subtract)
                yield None
                nc.vector.cody_waite_cascade(P[:], P[:], K[:], CW1, CW2, CW3)
                yield None
                E = epool.tile([EMB_X, FSUP], F32R, name="E")
                nc.scalar.activation(E[0:60, :], P[:], AF.Sin,
                                     bias=zeros_t[0:60, 0:1])
                nc.sync.dma_start(E[60:63, :], pts[:, sl].bitcast(F32R))
                # broadcast direction embedding to per-point
                embd = epool.tile([EMB_D, FSUP], F32R, name="embd")
                nc.sync.dma_start(
                    embd[:].rearrange("p (s r) -> p s r", s=S_SUP),
                    embd_rays[:].unsqueeze(1)
                    .broadcast_to([EMB_D, S_SUP, R_CORE]))
                yield (E, embd)

            def emb_first():
                """Super-tile 0 prologue: run the embedding pipeline in two
                column chunks so the PE can start layer 0 on the first
                512-point sub-tile ~5us earlier (element-wise identical)."""
                sl0 = slice(0, FSUP)
                P = spool.tile([60, FSUP], F32, name="P")
                K = spool.tile([60, FSUP], F32, name="K")
                for half in range(2):
                    for c in range(3):
                        nc.sync.dma_start(
                            P[half * 30 + c * 10:half * 30 + (c + 1) * 10, :],
                            pts[c:c + 1, sl0].partition_broadcast(10))
                E = epool.tile([EMB_X, FSUP], F32R, name="E")
                for a, b in ((0, F), (F, FSUP)):
                    Pv, Kv = P[:, a:b], K[:, a:b]
                    nc.vector.tensor_scalar(Pv, Pv, c_t[0:60, 0:1],
                                            c_t[0:60, 1:2],
                                            op0=ALU.mult, op1=ALU.add)
                    nc.vector.tensor_scalar(Kv, Pv, INV2PI, MAGIC,
                                            op0=ALU.mult, op1=ALU.add)
                    nc.vector.tensor_scalar(Kv, Kv, MAGIC, None,
                                            op0=ALU.subtract)
                    nc.vector.cody_waite_cascade(Pv, Pv, Kv, CW1, CW2, CW3)
                    nc.scalar.activation(E[0:60, a:b], Pv, AF.Sin,
                                         bias=zeros_t[0:60, 0:1])
                nc.sync.dma_start(E[60:63, :], pts[:, sl0].bitcast(F32R))
                return E

            E0 = emb_first()
            # ---- direction embedding per ray (once per core) ----
            for half in range(2):
                for c in range(3):
                    nc.sync.dma_start(
                        pd[half * 12 + c * 4:half * 12 + (c + 1) * 4, :],
                        dirs[c:c + 1, :].partition_broadcast(4))
            sincos(embd_rays, pd, kd, c_t[0:24, 2:4], 12)
            nc.sync.dma_start(embd_rays[24:27, :], dirs[:].bitcast(F32R))
            # st0 direction-embedding broadcast (must be emitted AFTER the
            # embd_rays writers: Tile tracks deps in emission order)
            embd0 = epool.tile([EMB_D, FSUP], F32R, name="embd")
            nc.sync.dma_start(
                embd0[:].rearrange("p (s r) -> p s r", s=S_SUP),
                embd_rays[:].unsqueeze(1)
                .broadcast_to([EMB_D, S_SUP, R_CORE]))
            emb_next = (E0, embd0)
            emb_gen = None
            for sti in range(nsup_exec):
                st = sti % NSUP
                sl = slice(st * FSUP, (st + 1) * FSUP)
                E, embd = emb_next
                emb_gen = (emb_stages((sti + 1) % NSUP)
                           if sti + 1 < nsup_exec else None)

                xa = apool.tile([128, 2 * FSUP], F32R, name="xa")
                xb = apool.tile([128, 2 * FSUP], F32R, name="xb")
                h = apool.tile([128, FSUP], F32R, name="h")
                osb = opool.tile([1, FSUP], F32, name="osb")
                rgbsb = opool.tile([3, FSUP], F32, name="rgbsb")

                def xsl(t, chunk, sub):
                    return t[:, chunk * FSUP + sub * F:
                             chunk * FSUP + sub * F + F]

                def rhs_of(src, idx, ksz, cur, sub):
                    if src == "E":
                        return E[0:EMB_X, sub * F:(sub + 1) * F]
                    if src == "embd":
                        return embd[0:EMB_D, sub * F:(sub + 1) * F]
                    return xsl(cur, idx, sub)

                cur = None
                # 8 xyz layers
                for li, kparts in enumerate(_XYZ_LAYERS):
                    nxt = xa if li % 2 == 0 else xb
                    for m in range(2):
                        lhs = []
                        for k, (src, idx, ksz) in enumerate(kparts):
                            if li == 0:
                                lt = w0_t[:, m * 128:(m + 1) * 128]
                            elif src == "E":
                                lt = w4e_t[:, m * 128:(m + 1) * 128]
                            else:
                                lt = wmid_t[li][:, idx * 256 + m * 128:
                                                idx * 256 + m * 128 + 128]
                            lhs.append(lt)
                        for sub in range(NSUB):
                            pt = psum.tile([128, F], F32, name="mmps",
                                           tag="mm")
                            for k, (src, idx, ksz) in enumerate(kparts):
                                nc.tensor.matmul(
                                    pt[:], lhs[k][0:ksz, :],
                                    rhs_of(src, idx, ksz, cur, sub),
                                    start=(k == 0),
                                    stop=(k == len(kparts) - 1))
                            evict(pt[:], xsl(nxt, m, sub),
                                  b_t[:, 2 * li + m:2 * li + m + 1])
                    cur = nxt
                    if emb_gen is not None and 1 <= li <= 5:
                        # emit one stage of the next super-tile's embedding
                        # pipeline so the in-order ACT/DVE queues interleave
                        # it with this tile's evictions instead of taking it
                        # as one burst; the final stage returns the tiles
                        r = next(emb_gen)
                        if r is not None:
                            emb_next = r

                # density head + feat layer (both read cur = x7)
                for sub in range(NSUB):
                    ptd = psum.tile([1, F], F32, name="denps", tag="mm")
                    for k in range(2):
                        nc.tensor.matmul(ptd[:], wden_t[:, k:k + 1],
                                         xsl(cur, k, sub),
                                         start=(k == 0), stop=(k == 1))
                    nc.scalar.activation(osb[0:1, sub * F:(sub + 1) * F],
                                         ptd[:], AF.Relu,
                                         bias=b_t[0:1, 19:20])
                nxt = xa if cur is xb else xb  # feat output
                for m in range(2):
                    for sub in range(NSUB):
                        pt = psum.tile([128, F], F32, name="featps", tag="mm")
                        for k in range(2):
                            nc.tensor.matmul(
                                pt[:],
                                wfeat_t[:, k * 256 + m * 128:
                                        k * 256 + m * 128 + 128],
                                xsl(cur, k, sub),
                                start=(k == 0), stop=(k == 1))
                        evict(pt[:], xsl(nxt, m, sub),
                              b_t[:, 16 + m:17 + m])
                cur = nxt

                # direction layer -> h
                for sub in range(NSUB):
                    pt = psum.tile([128, F], F32, name="dirps", tag="mm")
                    nc.tensor.matmul(pt[:], wd0_t[:, 0:128],
                                     xsl(cur, 0, sub), start=True, stop=False)
                    nc.tensor.matmul(pt[:], wd0_t[:, 128:256],
                                     xsl(cur, 1, sub), start=False, stop=False)
                    nc.tensor.matmul(pt[:], wd0e_t[:],
                                     embd[0:EMB_D, sub * F:(sub + 1) * F],
                                     start=False, stop=True)
                    evict(pt[:], h[:, sub * F:(sub + 1) * F],
                          b_t[:, 18:19])

                # rgb head
                for sub in range(NSUB):
                    ptr = psum.tile([3, F], F32, name="rgbps", tag="mm")
                    nc.tensor.matmul(ptr[:], wrgb_t[:],
                                     h[:, sub * F:(sub + 1) * F],
                                     start=True, stop=True)
                    nc.scalar.activation(rgbsb[:, sub * F:(sub + 1) * F],
                                         ptr[:], AF.Sigmoid,
                                         bias=b_t[0:3, 20:21])

                nc.sync.dma_start(out[0:1, sl], osb[:])
                nc.sync.dma_start(out[1:4, sl], rgbsb[:])

    nc.compile()
    _cache[key] = nc
    return nc


def _prep_inputs(inputs):
    """Host-side shard + transpose prep. Returns list of per-core dicts."""
    f32 = np.float32
    sp = np.ascontiguousarray(inputs["sample_points"], dtype=f32)
    dirs = np.ascontiguousarray(inputs["directions"], dtype=f32)

    dirs_all = dirs.T.copy()                          # [3, 4096]

    def wt(w):  # [out, in] -> [in, out]
        return np.ascontiguousarray(w.T, dtype=f32)

    def wmid_pack(w):  # [256, 256] -> [128, 512] (k-chunk blocks)
        t = wt(w)                                     # [256, 256]
        return np.ascontiguousarray(
            t.reshape(2, 128, 256).transpose(1, 0, 2).reshape(128, 512))

    shared = {}
    shared["w0"] = wt(inputs["Wx0"])                  # [63, 256]
    for i in range(1, 8):
        w = inputs[f"Wx{i}"]
        if i == 4:
            shared["wmid4"] = wmid_pack(w[:, :256])
            shared["w4e"] = wt(w[:, 256:])            # [63, 256]
        else:
            shared[f"wmid{i}"] = wmid_pack(w)
    shared["wfeat"] = wmid_pack(inputs["Wfeat"])
    wden_t = wt(inputs["Wden"])                       # [256, 1]
    shared["wden"] = np.ascontiguousarray(
        wden_t.reshape(2, 128, 1).transpose(1, 0, 2).reshape(128, 2))
    wd0_t = wt(inputs["Wd0"])                         # [283, 128]
    shared["wd0"] = np.ascontiguousarray(
        wd0_t[:256].reshape(2, 128, 128).transpose(1, 0, 2).reshape(128, 256))
    shared["wd0e"] = np.ascontiguousarray(wd0_t[256:])  # [27, 128]
    shared["wrgb"] = wt(inputs["Wrgb"])               # [128, 3]

    bias = np.zeros((128, 21), dtype=f32)
    for li in range(8):
        b = inputs[f"bx{li}"]
        bias[:, 2 * li] = b[:128]
        bias[:, 2 * li + 1] = b[128:]
    bias[:, 16] = inputs["bfeat"][:128]
    bias[:, 17] = inputs["bfeat"][128:]
    bias[:, 18] = inputs["bd0"]
    bias[0, 19] = inputs["bden"][0]
    bias[0:3, 20] = inputs["brgb"]

    consts = np.zeros((128, 4), dtype=f32)
    consts[0:30, 0] = 2.0 ** (np.arange(30) % 10)
    consts[30:60, 0] = 2.0 ** (np.arange(30) % 10)
    consts[30:60, 1] = np.pi / 2
    consts[0:12, 2] = 2.0 ** (np.arange(12) % 4)
    consts[12:24, 2] = 2.0 ** (np.arange(12) % 4)
    consts[12:24, 3] = np.pi / 2

    in_maps = []
    for c in range(N_CORES):
        m = dict(shared)
        # sample-major: [3, S, R] flattened to [3, NPTS]
        blk = sp[c * R_CORE:(c + 1) * R_CORE]         # [R, S, 3]
        m["pts"] = np.ascontiguousarray(
            blk.transpose(2, 1, 0).reshape(3, NPTS))
        m["dirs"] = np.ascontiguousarray(
            dirs_all[:, c * R_CORE:(c + 1) * R_CORE])
        m["biases"] = bias
        m["consts"] = consts
        in_maps.append(m)
    return in_maps


def kernel(**inputs) -> np.ndarray:
    nc = _build()
    in_maps = _prep_inputs(inputs)
    res = run_bass_kernel_spmd(nc, in_maps, core_ids=list(range(N_CORES)))
    outs = []
    for c in range(N_CORES):
        o = res.results[c]["out"]                     # [4, NPTS] sample-major
        outs.append(o.reshape(4, S, R_CORE).transpose(2, 1, 0))
    return np.concatenate(outs, axis=0)



# revision 8
# speedup vs baseline: 1.2677x; 1.2677x over previous
"""NeRF MLP forward pass on 8 Trainium2 NeuronCores (Bass/Tile).

Strategy: pure data parallel over rays. Each core processes 512 rays x 64
samples = 32768 points through the full MLP. Activations live transposed in
SBUF as [hidden, n_points]. The seven 256->256 hidden layers and the
direction layer's 256-wide part run as fp8(e4m3) DoubleRow matmuls
(K=256 per instruction, half-rate per output column); the input layer,
skip-connection part, density/feature/rgb heads stay in float32r for
accuracy. Harmonic embeddings are computed on-chip with magic-number
round-to-nearest range reduction + the ScalarE Sin LUT. The 1-wide density
and 3-wide rgb heads are packed 4 sub-tiles deep into the partition dim via
PE tile_position column offsets so their evictions cost 512 columns, not
2048.
"""

import sys

if '/opt/trn_rl_repo' not in sys.path:
    sys.path.insert(0, '/opt/trn_rl_repo')

import ml_dtypes
import numpy as np

import concourse.bacc as bacc
import concourse.mybir as mybir
import concourse.tile as tile
from concourse.bass_utils import run_bass_kernel_spmd

F32 = mybir.dt.float32
F32R = mybir.dt.float32r
F8 = mybir.dt.float8e4
BF16 = mybir.dt.bfloat16
NP_F8 = ml_dtypes.float8_e4m3
NP_BF16 = ml_dtypes.bfloat16
AF = mybir.ActivationFunctionType
ALU = mybir.AluOpType
DR = mybir.MatmulPerfMode.DoubleRow

N_CORES = 8
N_RAYS, S = 4096, 64
R_CORE = N_RAYS // N_CORES            # 512 rays per core
NPTS = R_CORE * S                     # 32768 points per core
# Points are ordered SAMPLE-major per core: point index = s * R_CORE + r.
F = 512                               # points per matmul (one PSUM bank)
FSUP = 2048                           # points per super-tile
NSUB = FSUP // F                      # 4
NSUP = NPTS // FSUP                   # 16
S_SUP = FSUP // R_CORE                # 4 samples per super-tile

H = 256
EMB_X = 63
EMB_D = 27

PI = float(np.pi)
INV2PI = float(1.0 / (2.0 * np.pi))
TWOPI = float(2.0 * np.pi)
MAGIC = float(1.5 * 2 ** 23)
# Cody-Waite split of 2*pi (used only for the per-ray direction sincos).
_t = 2.0 * np.pi - 6.28125
_c2u = np.float32(_t).view(np.uint32) & np.uint32(0xFFFFF000)
CW1 = 6.28125
CW2 = float(_c2u.view(np.float32))
CW3 = float(np.float32(_t - float(_c2u.view(np.float32))))

_cache = {}


def _build(nsup_exec=NSUP):
    """Build the bass program. nsup_exec > NSUP repeats super-tiles
    (st = i % NSUP) — used only for slope-based timing benchmarks."""
    key = ("nc", nsup_exec)
    if key in _cache:
        return _cache[key]

    nc = bacc.Bacc("TRN2", target_bir_lowering=False, debug=False,
                   num_devices=N_CORES)

    pts = nc.dram_tensor("pts", [3, NPTS], F32, kind="ExternalInput")
    dirs = nc.dram_tensor("dirs", [3, R_CORE], F32, kind="ExternalInput")
    w0 = nc.dram_tensor("w0", [EMB_X, 256], F32, kind="ExternalInput")
    w8mid = {i: nc.dram_tensor(f"w8mid{i}", [128, 512], F8,
                               kind="ExternalInput")
             for i in range(1, 8)}
    w4e = nc.dram_tensor("w4e", [EMB_X, 256], F32, kind="ExternalInput")
    wfeat = nc.dram_tensor("wfeat", [128, 512], BF16, kind="ExternalInput")
    wden = nc.dram_tensor("wden", [128, 64], BF16, kind="ExternalInput")
    wd0x = nc.dram_tensor("wd0x", [128, 256], F8, kind="ExternalInput")
    wd0e = nc.dram_tensor("wd0e", [EMB_D, 128], F32, kind="ExternalInput")
    wrgb = nc.dram_tensor("wrgb", [128, 32], BF16, kind="ExternalInput")
    biases = nc.dram_tensor("biases", [128, 21], F32, kind="ExternalInput")
    consts = nc.dram_tensor("consts", [128, 4], F32, kind="ExternalInput")
    out = nc.dram_tensor("out", [4, NPTS], F32, kind="ExternalOutput")

    with tile.TileContext(nc) as tc:
        with (
            tc.tile_pool(name="wpool", bufs=1) as wpool,
            tc.tile_pool(name="epool", bufs=3) as epool,
            tc.tile_pool(name="spool", bufs=2) as spool,
            tc.tile_pool(name="apool", bufs=1) as apool,
            tc.tile_pool(name="opool", bufs=2) as opool,
            tc.tile_pool(name="psum", bufs=3, space="PSUM") as psum,
            tc.tile_pool(name="psumh", bufs=1, space="PSUM") as psumh,
        ):
            # ---- load weights / constants (once) ----
            w0_t = wpool.tile([EMB_X, 256], F32R)
            nc.sync.dma_start(w0_t[:], w0[:].bitcast(F32R))
            w8_t = {}
            for i in range(1, 8):
                wt = wpool.tile([128, 512], F8, name=f"w8mid{i}_t")
                nc.sync.dma_start(wt[:], w8mid[i][:])
                w8_t[i] = wt
            w4e_t = wpool.tile([EMB_X, 256], F32R)
            nc.sync.dma_start(w4e_t[:], w4e[:].bitcast(F32R))
            wfeat_t = wpool.tile([128, 512], BF16)
            nc.sync.dma_start(wfeat_t[:], wfeat[:])
            wden_t = wpool.tile([128, 64], BF16)
            nc.sync.dma_start(wden_t[:], wden[:])
            wd0x_t = wpool.tile([128, 256], F8)
            nc.sync.dma_start(wd0x_t[:], wd0x[:])
            wd0e_t = wpool.tile([EMB_D, 128], F32R)
            nc.sync.dma_start(wd0e_t[:], wd0e[:].bitcast(F32R))
            wrgb_t = wpool.tile([128, 32], BF16)
            nc.sync.dma_start(wrgb_t[:], wrgb[:])
            b_t = wpool.tile([128, 21], F32)
            nc.sync.dma_start(b_t[:], biases[:])
            c_t = wpool.tile([128, 4], F32)
            nc.sync.dma_start(c_t[:], consts[:])
            zeros_t = wpool.tile([128, 1], F32)
            nc.any.memset(zeros_t[:], 0.0)

            def w8ap(t):
                return t[:].rearrange("p (j m) -> p j m", j=2)

            # ---- per-ray direction embedding (once per core) ----
            embd_rays = wpool.tile([EMB_D, R_CORE], F32R)
            pd = wpool.tile([24, R_CORE], F32)
            kd = wpool.tile([24, R_CORE], F32)

            # running engine-busy estimates for eviction load balancing (ns)
            load = {"act": 0.0, "dve": 0.0}

            def evict(psum_ap, out_ap, bias_ap, ncols, func=AF.Relu):
                """relu(psum + bias) -> SBUF, on the least-loaded of ACT/DVE."""
                cost_act = ncols * 0.8333 + 185.0
                cost_dve = ncols * 1.0417 + 125.0
                if load["act"] + cost_act <= load["dve"] + cost_dve:
                    load["act"] += cost_act
                    nc.scalar.activation(out_ap, psum_ap, func, bias=bias_ap)
                else:
                    load["dve"] += cost_dve
                    nc.vector.tensor_scalar(out_ap, psum_ap, bias_ap, 0.0,
                                            op0=ALU.add, op1=ALU.max)

            def act_fixed(cost):
                load["act"] += cost

            def dve_fixed(cost):
                load["dve"] += cost

            def emb_stages(st):
                """Generator emitting the harmonic-embedding pipeline for
                super-tile st one stage per next() call. Final yield returns
                (E, embd).

                args = x * (2^h / 2pi) + phi  (phi = 0.25 for the cos rows)
                Km   = fl(-args - MAGIC) = -MAGIC - round(args)
                frac = (Km + MAGIC) + args = args - round(args)  in [-.5, .5]
                E    = Sin(frac * 2pi)
                """
                sl = slice(st * FSUP, (st + 1) * FSUP)
                P = spool.tile([60, FSUP], F32, name="P")
                K = spool.tile([60, FSUP], F32, name="K")
                for half in range(2):
                    for c in range(3):
                        nc.sync.dma_start(
                            P[half * 30 + c * 10:half * 30 + (c + 1) * 10, :],
                            pts[c:c + 1, sl].partition_broadcast(10))
                nc.gpsimd.tensor_scalar(P[:], P[:], c_t[0:60, 0:1],
                                        c_t[0:60, 1:2],
                                        op0=ALU.mult, op1=ALU.add)
                yield None
                nc.gpsimd.tensor_scalar(K[:], P[:], -1.0, -MAGIC,
                                        op0=ALU.mult, op1=ALU.add)
                yield None
                nc.vector.affine_then_add(P[:], K[:], P[:], 1.0, MAGIC)
                dve_fixed(FSUP * 1.0417 + 125.0)
                yield None
                E = epool.tile([EMB_X, FSUP], F32R, name="E")
                nc.scalar.activation(E[0:60, :], P[:], AF.Sin,
                                     bias=zeros_t[0:60, 0:1], scale=TWOPI)
                act_fixed(FSUP * 0.8333 + 185.0)
                nc.sync.dma_start(E[60:63, :], pts[:, sl].bitcast(F32R))
                # broadcast direction embedding to per-point
                embd = epool.tile([EMB_D, FSUP], F32R, name="embd")
                nc.sync.dma_start(
                    embd[:].rearrange("p (s r) -> p s r", s=S_SUP),
                    embd_rays[:].unsqueeze(1)
                    .broadcast_to([EMB_D, S_SUP, R_CORE]))
                yield (E, embd)

            def emb_first():
                """Super-tile 0 prologue: run the embedding pipeline in two
                column chunks so the PE can start layer 0 earlier."""
                sl0 = slice(0, FSUP)
                P = spool.tile([60, FSUP], F32, name="P")
                K = spool.tile([60, FSUP], F32, name="K")
                for half in range(2):
                    for c in range(3):
                        nc.sync.dma_start(
                            P[half * 30 + c * 10:half * 30 + (c + 1) * 10, :],
                            pts[c:c + 1, sl0].partition_broadcast(10))
                E = epool.tile([EMB_X, FSUP], F32R, name="E")
                for a, b in ((0, F), (F, FSUP)):
                    Pv, Kv = P[:, a:b], K[:, a:b]
                    nc.vector.tensor_scalar(Pv, Pv, c_t[0:60, 0:1],
                                            c_t[0:60, 1:2],
                                            op0=ALU.mult, op1=ALU.add)
                    nc.vector.tensor_scalar(Kv, Pv, -1.0, -MAGIC,
                                            op0=ALU.mult, op1=ALU.add)
                    nc.vector.affine_then_add(Pv, Kv, Pv, 1.0, MAGIC)
                    nc.scalar.activation(E[0:60, a:b], Pv, AF.Sin,
                                         bias=zeros_t[0:60, 0:1], scale=TWOPI)
                nc.sync.dma_start(E[60:63, :], pts[:, sl0].bitcast(F32R))
                return E

            def sincos_dirs():
                """Per-ray [27, R_CORE] direction embedding via Cody-Waite."""
                for half in range(2):
                    for c in range(3):
                        nc.sync.dma_start(
                            pd[half * 12 + c * 4:half * 12 + (c + 1) * 4, :],
                            dirs[c:c + 1, :].partition_broadcast(4))
                nc.vector.tensor_scalar(pd[:], pd[:], c_t[0:24, 2:3],
                                        c_t[0:24, 3:4],
                                        op0=ALU.mult, op1=ALU.add)
                nc.vector.tensor_scalar(kd[:], pd[:], INV2PI, MAGIC,
                                        op0=ALU.mult, op1=ALU.add)
                nc.vector.tensor_scalar(kd[:], kd[:], MAGIC, None,
                                        op0=ALU.subtract)
                nc.vector.cody_waite_cascade(pd[:], pd[:], kd[:],
                                             CW1, CW2, CW3)
                nc.scalar.activation(embd_rays[0:24, :], pd[:], AF.Sin,
                                     bias=zeros_t[0:24, 0:1])
                nc.sync.dma_start(embd_rays[24:27, :], dirs[:].bitcast(F32R))

            E0 = emb_first()
            sincos_dirs()
            # st0 direction-embedding broadcast (after the embd_rays writers)
            embd0 = epool.tile([EMB_D, FSUP], F32R, name="embd")
            nc.sync.dma_start(
                embd0[:].rearrange("p (s r) -> p s r", s=S_SUP),
                embd_rays[:].unsqueeze(1)
                .broadcast_to([EMB_D, S_SUP, R_CORE]))

            emb_next = (E0, embd0)
            emb_gen = None
            for sti in range(nsup_exec):
                st = sti % NSUP
                sl = slice(st * FSUP, (st + 1) * FSUP)
                E, embd = emb_next
                emb_gen = (emb_stages((sti + 1) % NSUP)
                           if sti + 1 < nsup_exec else None)

                xa8 = apool.tile([128, 2 * FSUP], F8, name="xa8")
                xb8 = apool.tile([128, 2 * FSUP], F8, name="xb8")
                x7f = apool.tile([128, 2 * FSUP], BF16, name="x7f")
                ft8 = apool.tile([128, 2 * FSUP], F8, name="ft8")
                hf = apool.tile([128, FSUP], BF16, name="hf")
                osb = opool.tile([128, F], F32, name="osb")
                rgbsb = opool.tile([128, F], F32, name="rgbsb")

                def pair_ap(t):
                    return t[:].rearrange("p (j n) -> p j n", j=2)

                def xsl(t, chunk, a, b):
                    return t[:, chunk * FSUP + a:chunk * FSUP + b]

                def step_emb():
                    if emb_gen is None:
                        return None
                    r = next(emb_gen)
                    return r

                # ---- L0: fp32r from E -> xa8 ----
                for m in range(2):
                    for sp in range(2):
                        pt = psum.tile([128, 2 * F], F32, name="mmps",
                                       tag="mm")
                        for s2 in range(2):
                            sub = sp * 2 + s2
                            nc.tensor.matmul(
                                pt[:, s2 * F:(s2 + 1) * F],
                                w0_t[:, m * 128:(m + 1) * 128],
                                E[0:EMB_X, sub * F:(sub + 1) * F],
                                start=True, stop=True)
                        evict(pt[:], xsl(xa8, m, sp * 2 * F, (sp + 1) * 2 * F),
                              b_t[:, m:m + 1], 2 * F)

                cur = xa8
                # ---- L1..L7 ----
                for li in range(1, 8):
                    nxt = xb8 if cur is xa8 else xa8
                    if li == 7:
                        nxt = x7f
                    w3 = w8ap(w8_t[li])
                    cur3 = pair_ap(cur)
                    for sp in range(2):
                        for m in range(2):
                            pt = psum.tile([128, 2 * F], F32, name="mmps",
                                           tag="mm")
                            for s2 in range(2):
                                sub = sp * 2 + s2
                                last = li != 4
                                nc.tensor.matmul(
                                    pt[:, s2 * F:(s2 + 1) * F],
                                    w3[:, :, m * 128:(m + 1) * 128],
                                    cur3[:, :, sub * F:(sub + 1) * F],
                                    start=True, stop=last, perf_mode=DR)
                                if li == 4:
                                    nc.tensor.matmul(
                                        pt[:, s2 * F:(s2 + 1) * F],
                                        w4e_t[:, m * 128:(m + 1) * 128],
                                        E[0:EMB_X, sub * F:(sub + 1) * F],
                                        start=False, stop=True)
                            evict(pt[:],
                                  xsl(nxt, m, sp * 2 * F, (sp + 1) * 2 * F),
                                  b_t[:, 2 * li + m:2 * li + m + 1], 2 * F)
                    cur = nxt
                    if 1 <= li <= 3 or li == 5:
                        r = step_emb()
                        if r is not None:
                            emb_next = r

                # ---- den (fp32r, M=32-padded, partition-packed) + rgb psum
                drps = psumh.tile([128, 2 * F], F32, name="drps")
                for sub in range(NSUB):
                    for k in range(2):
                        nc.tensor.matmul(
                            drps[32 * sub:32 * sub + 32, 0:F],
                            wden_t[:, k * 32:(k + 1) * 32],
                            xsl(x7f, k, sub * F, (sub + 1) * F),
                            start=(k == 0), stop=(k == 1),
                            tile_position=(0, 32 * sub))

                # ---- feat (fp32r) -> ft8 ----
                for sp in range(2):
                    for m in range(2):
                        pt = psum.tile([128, 2 * F], F32, name="mmps",
                                       tag="mm")
                        for s2 in range(2):
                            sub = sp * 2 + s2
                            for k in range(2):
                                nc.tensor.matmul(
                                    pt[:, s2 * F:(s2 + 1) * F],
                                    wfeat_t[:, k * 256 + m * 128:
                                            k * 256 + m * 128 + 128],
                                    xsl(x7f, k, sub * F, (sub + 1) * F),
                                    start=(k == 0), stop=(k == 1))
                        evict(pt[:], xsl(ft8, m, sp * 2 * F, (sp + 1) * 2 * F),
                              b_t[:, 16 + m:17 + m], 2 * F)

                # ---- dir layer: DR(ft8) + fp32r(embd) -> hf ----
                ft3 = pair_ap(ft8)
                for sp in range(2):
                    pt = psum.tile([128, 2 * F], F32, name="mmps", tag="mm")
                    for s2 in range(2):
                        sub = sp * 2 + s2
                        nc.tensor.matmul(
                            pt[:, s2 * F:(s2 + 1) * F],
                            w8ap(wd0x_t)[:, :, 0:128],
                            ft3[:, :, sub * F:(sub + 1) * F],
                            start=True, stop=False, perf_mode=DR)
                        nc.tensor.matmul(
                            pt[:, s2 * F:(s2 + 1) * F],
                            wd0e_t[:],
                            embd[0:EMB_D, sub * F:(sub + 1) * F],
                            start=False, stop=True)
                    evict(pt[:], hf[:, sp * 2 * F:(sp + 1) * 2 * F],
                          b_t[:, 18:19], 2 * F)

                # ---- rgb (fp32r, M=32-padded, partition-packed) ----
                for sub in range(NSUB):
                    nc.tensor.matmul(
                        drps[32 * sub:32 * sub + 32, F:2 * F],
                        wrgb_t[:],
                        hf[:, sub * F:(sub + 1) * F],
                        start=True, stop=True,
                        tile_position=(0, 32 * sub))

                # den eviction: rows {0,32,64,96} are the real density
                nc.vector.tensor_scalar(osb[:], drps[:, 0:F],
                                        b_t[:, 19:20], 0.0,
                                        op0=ALU.add, op1=ALU.max)
                dve_fixed(F * 1.0417 + 125.0)
                # sigmoid(z) = 0.5 + 0.5*tanh(z/2): Tanh lives in the same
                # activation table as Sin (silu_and_others), so the ACT
                # engine never swaps tables; the final affine runs on GPSIMD.
                nc.scalar.activation(rgbsb[:], drps[:, F:2 * F],
                                     AF.Tanh, bias=b_t[:, 20:21], scale=0.5)
                act_fixed(F * 0.8333 + 185.0)
                nc.gpsimd.tensor_scalar(rgbsb[:], rgbsb[:], 0.5, 0.5,
                                        op0=ALU.mult, op1=ALU.add)

                nc.sync.dma_start(
                    out[0:1, sl].rearrange("o (s n) -> (o s) n", s=NSUB),
                    osb[0:128:32, :])
                for c in range(3):
                    nc.sync.dma_start(
                        out[1 + c:2 + c, sl]
                        .rearrange("o (s n) -> (o s) n", s=NSUB),
                        rgbsb[c:128:32, :])

    _patch_act_table_loads(nc)
    nc.compile()
    _cache[key] = nc
    return nc


def _patch_act_table_loads(nc):
    """Every ACT func used here (Sin, Relu, Tanh, Identity) lives in the
    silu_and_others table, but the stock insert_act_table_loads pass picks
    tables greedily first-match (trig_and_small for Sin, exp_and_others for
    Tanh) and thrashes 2 loads x 1.3us per super-tile. Place one load of
    silu_and_others before the first activation instead (same contract:
    loads pre-placed on the Bass CFG, walrus adopts them)."""
    import types

    from concourse.hw_specs import get_activation_tables

    def _single_load(self):
        tabs = get_activation_tables(self.m.arch)
        names = list(tabs)
        idx = names.index("silu_and_others")
        allowed = tabs["silu_and_others"]
        for blk in self.main_func.blocks:
            for i, inst in enumerate(blk.instructions):
                if isinstance(inst, mybir.InstActivation):
                    assert inst.func in allowed, (
                        f"activation {inst.func} not in silu_and_others; "
                        f"single-table-load patch is invalid")
        for blk in self.main_func.blocks:
            for i, inst in enumerate(blk.instructions):
                if isinstance(inst, mybir.InstActivation):
                    load = mybir.InstLoadActFuncSet(
                        name=self.get_next_instruction_name(),
                        act_func_set_id=idx, ins=[], outs=[])
                    load.engine = mybir.EngineType.Activation
                    self.register_instruction(load)
                    blk.instructions.insert(i, load)
                    return

    nc.insert_act_table_loads = types.MethodType(_single_load, nc)


def _prep_inputs(inputs):
    """Host-side shard + transpose prep. Returns list of per-core dicts."""
    f32 = np.float32
    sp = np.ascontiguousarray(inputs["sample_points"], dtype=f32)
    dirs = np.ascontiguousarray(inputs["directions"], dtype=f32)

    dirs_all = dirs.T.copy()                          # [3, 4096]

    def wt(w):  # [out, in] -> [in, out]
        return np.ascontiguousarray(w.T, dtype=f32)

    def kpack(w, m):  # [m, 2*128k] -> [128, 2, m] -> [128, 2*m]
        t = wt(w)                                     # [in, m]
        kin = t.shape[0] // 2
        assert kin == 128
        return np.ascontiguousarray(
            t.reshape(2, 128, m).transpose(1, 0, 2).reshape(128, 2 * m))

    shared = {}
    shared["w0"] = wt(inputs["Wx0"])                  # [63, 256]
    for i in range(1, 8):
        w = inputs[f"Wx{i}"]
        if i == 4:
            shared["w8mid4"] = kpack(w[:, :256], 256).astype(NP_F8)
            shared["w4e"] = wt(w[:, 256:])            # [63, 256]
        else:
            shared[f"w8mid{i}"] = kpack(w, 256).astype(NP_F8)
    shared["wfeat"] = kpack(inputs["Wfeat"], 256).astype(NP_BF16)
    wden_pad = np.zeros((32, 256), dtype=f32)
    wden_pad[0] = inputs["Wden"][0]
    shared["wden"] = kpack(wden_pad, 32).astype(NP_BF16)  # [128, 64]
    wd0 = inputs["Wd0"]                               # [128, 283]
    shared["wd0x"] = kpack(wd0[:, :256], 128).astype(NP_F8)
    shared["wd0e"] = np.ascontiguousarray(wt(wd0)[256:])  # [27, 128]
    wrgb_pad = np.zeros((32, 128), dtype=f32)
    wrgb_pad[0:3] = inputs["Wrgb"]
    shared["wrgb"] = wt(wrgb_pad).astype(NP_BF16)     # [128, 32]

    bias = np.zeros((128, 21), dtype=f32)
    for li in range(8):
        b = inputs[f"bx{li}"]
        bias[:, 2 * li] = b[:128]
        bias[:, 2 * li + 1] = b[128:]
    bias[:, 16] = inputs["bfeat"][:128]
    bias[:, 17] = inputs["bfeat"][128:]
    bias[:, 18] = inputs["bd0"]
    for s in range(4):
        bias[32 * s, 19] = inputs["bden"][0]
        # rgb eviction computes tanh(psum*0.5 + brgb*0.5)
        bias[32 * s:32 * s + 3, 20] = inputs["brgb"] * 0.5

    consts = np.zeros((128, 4), dtype=f32)
    consts[0:60, 0] = 2.0 ** (np.arange(60) % 30 % 10) / (2 * np.pi)
    consts[30:60, 1] = 0.25
    consts[0:24, 2] = 2.0 ** (np.arange(24) % 12 % 4)
    consts[12:24, 3] = np.pi / 2

    in_maps = []
    for c in range(N_CORES):
        m = dict(shared)
        # sample-major: [3, S, R] flattened to [3, NPTS]
        blk = sp[c * R_CORE:(c + 1) * R_CORE]         # [R, S, 3]
        m["pts"] = np.ascontiguousarray(
            blk.transpose(2, 1, 0).reshape(3, NPTS))
        m["dirs"] = np.ascontiguousarray(
            dirs_all[:, c * R_CORE:(c + 1) * R_CORE])
        m["biases"] = bias
        m["consts"] = consts
        in_maps.append(m)
    return in_maps


def kernel(**inputs) -> np.ndarray:
    nc = _build()
    in_maps = _prep_inputs(inputs)
    res = run_bass_kernel_spmd(nc, in_maps, core_ids=list(range(N_CORES)))
    outs = []
    for c in range(N_CORES):
        o = res.results[c]["out"]                     # [4, NPTS] sample-major
        outs.append(o.reshape(4, S, R_CORE).transpose(2, 1, 0))
    return np.concatenate(outs, axis=0)


# revision 15
# speedup vs baseline: 1.3759x; 1.0854x over previous
"""NeRF MLP forward pass on 8 Trainium2 NeuronCores (Bass/Tile).

Strategy: pure data parallel over rays. Each core processes 512 rays x 64
samples = 32768 points through the full MLP. Activations live transposed in
SBUF as [hidden, n_points]. The seven 256->256 hidden layers and the
direction layer's 256-wide part run as fp8(e4m3) DoubleRow matmuls
(K=256 per instruction, half-rate per output column); the input layer,
skip-connection part, density/feature/rgb heads stay in float32r for
accuracy. Harmonic embeddings are computed on-chip with magic-number
round-to-nearest range reduction + the ScalarE Sin LUT. The 1-wide density
and 3-wide rgb heads are packed 4 sub-tiles deep into the partition dim via
PE tile_position column offsets so their evictions cost 512 columns, not
2048.
"""

import sys

if '/opt/trn_rl_repo' not in sys.path:
    sys.path.insert(0, '/opt/trn_rl_repo')

import ml_dtypes
import numpy as np

import concourse.bacc as bacc
import concourse.mybir as mybir
import concourse.tile as tile
from concourse.bass_utils import run_bass_kernel_spmd

F32 = mybir.dt.float32
F32R = mybir.dt.float32r
F8 = mybir.dt.float8e4
BF16 = mybir.dt.bfloat16
NP_F8 = ml_dtypes.float8_e4m3
NP_BF16 = ml_dtypes.bfloat16
AF = mybir.ActivationFunctionType
ALU = mybir.AluOpType
DR = mybir.MatmulPerfMode.DoubleRow

N_CORES = 8
N_RAYS, S = 4096, 64
R_CORE = N_RAYS // N_CORES            # 512 rays per core
NPTS = R_CORE * S                     # 32768 points per core
# Points are ordered SAMPLE-major per core: point index = s * R_CORE + r.
F = 512                               # points per matmul (one PSUM bank)
FSUP = 2048                           # points per super-tile
NSUB = FSUP // F                      # 4
NSUP = NPTS // FSUP                   # 16
S_SUP = FSUP // R_CORE                # 4 samples per super-tile

H = 256
EMB_X = 63
EMB_D = 27

PI = float(np.pi)
INV2PI = float(1.0 / (2.0 * np.pi))
TWOPI = float(2.0 * np.pi)
MAGIC = float(1.5 * 2 ** 23)
# Cody-Waite split of 2*pi (used only for the per-ray direction sincos).
_t = 2.0 * np.pi - 6.28125
_c2u = np.float32(_t).view(np.uint32) & np.uint32(0xFFFFF000)
CW1 = 6.28125
CW2 = float(_c2u.view(np.float32))
CW3 = float(np.float32(_t - float(_c2u.view(np.float32))))

_cache = {}


def _build(nsup_exec=NSUP):
    """Build the bass program. nsup_exec > NSUP repeats super-tiles
    (st = i % NSUP) — used only for slope-based timing benchmarks."""
    key = ("nc", nsup_exec)
    if key in _cache:
        return _cache[key]

    nc = bacc.Bacc("TRN2", target_bir_lowering=False, debug=False,
                   num_devices=N_CORES)

    pts = nc.dram_tensor("pts", [3, NPTS], F32, kind="ExternalInput")
    dirs = nc.dram_tensor("dirs", [3, R_CORE], F32, kind="ExternalInput")
    w0 = nc.dram_tensor("w0", [EMB_X, 256], F32, kind="ExternalInput")
    w8mid = nc.dram_tensor("w8mid", [128, 7 * 512], F8,
                           kind="ExternalInput")
    w4e = nc.dram_tensor("w4e", [EMB_X, 256], F32, kind="ExternalInput")
    wfeat = nc.dram_tensor("wfeat", [128, 512], BF16, kind="ExternalInput")
    wden = nc.dram_tensor("wden", [128, 64], BF16, kind="ExternalInput")
    wd0x = nc.dram_tensor("wd0x", [128, 256], F8, kind="ExternalInput")
    wd0e = nc.dram_tensor("wd0e", [EMB_D, 128], F32, kind="ExternalInput")
    wrgb = nc.dram_tensor("wrgb", [128, 32], BF16, kind="ExternalInput")
    biases = nc.dram_tensor("biases", [128, 21], F32, kind="ExternalInput")
    consts = nc.dram_tensor("consts", [128, 4], F32, kind="ExternalInput")
    out = nc.dram_tensor("out", [4, NPTS], F32, kind="ExternalOutput")

    with tile.TileContext(nc) as tc:
        with (
            tc.tile_pool(name="wpool", bufs=1) as wpool,
            tc.tile_pool(name="epool", bufs=3) as epool,
            tc.tile_pool(name="spool", bufs=3) as spool,
            tc.tile_pool(name="apool", bufs=1) as apool,
            tc.tile_pool(name="opool", bufs=3) as opool,
            tc.tile_pool(name="psum", bufs=4, space="PSUM") as psum,
        ):
            # ---- prologue DMAs first: st0 point/dir broadcasts, then the
            # constants and weights in consumption order (w0 before the rest)
            P0 = spool.tile([60, FSUP], F32, name="P")
            K0 = spool.tile([60, FSUP], F32, name="K")
            for c in range(3):
                nc.sync.dma_start(P0[c * 20:(c + 1) * 20, :],
                                  pts[c:c + 1, 0:FSUP].partition_broadcast(20))
            pd = wpool.tile([24, R_CORE], F32)
            kd = wpool.tile([24, R_CORE], F32)
            for c in range(3):
                nc.sync.dma_start(pd[c * 8:(c + 1) * 8, :],
                                  dirs[c:c + 1, :].partition_broadcast(8))
            b_t = wpool.tile([128, 21], F32)
            nc.sync.dma_start(b_t[:], biases[:])
            c_t = wpool.tile([128, 4], F32)
            nc.sync.dma_start(c_t[:], consts[:])
            zeros_t = wpool.tile([128, 1], F32)
            nc.any.memset(zeros_t[:], 0.0)

            # ---- load weights (once) ----
            w0_t = wpool.tile([EMB_X, 256], F32R)
            nc.sync.dma_start(w0_t[:], w0[:].bitcast(F32R))
            w8all_t = wpool.tile([128, 7 * 512], F8)
            nc.sync.dma_start(w8all_t[:], w8mid[:])
            w8_t = {i: w8all_t[:, (i - 1) * 512:i * 512] for i in range(1, 8)}
            w4e_t = wpool.tile([EMB_X, 256], F32R)
            nc.sync.dma_start(w4e_t[:], w4e[:].bitcast(F32R))
            wfeat_t = wpool.tile([128, 512], BF16)
            nc.sync.dma_start(wfeat_t[:], wfeat[:])
            wden_t = wpool.tile([128, 64], BF16)
            nc.sync.dma_start(wden_t[:], wden[:])
            wd0x_t = wpool.tile([128, 256], F8)
            nc.sync.dma_start(wd0x_t[:], wd0x[:])
            wd0e_t = wpool.tile([EMB_D, 128], F32R)
            nc.sync.dma_start(wd0e_t[:], wd0e[:].bitcast(F32R))
            wrgb_t = wpool.tile([128, 32], BF16)
            nc.sync.dma_start(wrgb_t[:], wrgb[:])

            def w8ap(t):
                if hasattr(t, 'rearrange'):
                    return t.rearrange("p (j m) -> p j m", j=2)
                return t[:].rearrange("p (j m) -> p j m", j=2)

            # ---- per-ray direction embedding (once per core) ----
            embd_rays = wpool.tile([EMB_D, R_CORE], F32R)

            # running engine-busy estimates for eviction load balancing (ns)
            load = {"act": 0.0, "dve": 0.0}

            def evict(psum_ap, out_ap, bias_ap, ncols, func=AF.Relu):
                """relu(psum + bias) -> SBUF, on the least-loaded of ACT/DVE."""
                cost_act = ncols * 0.8333 + 185.0
                cost_dve = ncols * 1.0417 + 125.0
                if load["act"] + cost_act <= load["dve"] + cost_dve:
                    load["act"] += cost_act
                    nc.scalar.activation(out_ap, psum_ap, func, bias=bias_ap)
                else:
                    load["dve"] += cost_dve
                    nc.vector.tensor_scalar(out_ap, psum_ap, bias_ap, 0.0,
                                            op0=ALU.add, op1=ALU.max)

            def act_fixed(cost):
                load["act"] += cost

            def dve_fixed(cost):
                load["dve"] += cost

            def emb_stages(st):
                """Generator emitting the harmonic-embedding pipeline for
                super-tile st one stage per next() call. Final yield returns
                (E, embd).

                args = x * (2^h / 2pi) + phi  (phi = 0.25 for the cos rows)
                Km   = fl(-args - MAGIC) = -MAGIC - round(args)
                frac = (Km + MAGIC) + args = args - round(args)  in [-.5, .5]
                E    = Sin(frac * 2pi)
                """
                sl = slice(st * FSUP, (st + 1) * FSUP)
                P = spool.tile([60, FSUP], F32, name="P")
                K = spool.tile([60, FSUP], F32, name="K")
                for c in range(3):
                    nc.sync.dma_start(
                        P[c * 20:(c + 1) * 20, :],
                        pts[c:c + 1, sl].partition_broadcast(20))
                nc.gpsimd.tensor_scalar(P[:], P[:], c_t[0:60, 0:1],
                                        c_t[0:60, 1:2],
                                        op0=ALU.mult, op1=ALU.add)
                yield None
                nc.gpsimd.tensor_scalar(K[:], P[:], -1.0, -MAGIC,
                                        op0=ALU.mult, op1=ALU.add)
                yield None
                nc.gpsimd.tensor_scalar(K[:], K[:], MAGIC, None, op0=ALU.add)
                yield None
                nc.gpsimd.tensor_tensor(P[:], K[:], P[:], op=ALU.add)
                yield None
                E = epool.tile([EMB_X, FSUP], F32R, name="E")
                nc.scalar.activation(E[0:60, 0:2 * F], P[:, 0:2 * F], AF.Sin,
                                     bias=zeros_t[0:60, 0:1], scale=TWOPI)
                act_fixed(2 * F * 0.8333 + 185.0)
                yield None
                nc.scalar.activation(E[0:60, 2 * F:], P[:, 2 * F:], AF.Sin,
                                     bias=zeros_t[0:60, 0:1], scale=TWOPI)
                act_fixed(2 * F * 0.8333 + 185.0)
                nc.sync.dma_start(E[60:63, :], pts[:, sl].bitcast(F32R))
                # broadcast direction embedding to per-point
                embd = epool.tile([EMB_D, FSUP], F32R, name="embd")
                nc.sync.dma_start(
                    embd[:].rearrange("p (s r) -> p s r", s=S_SUP),
                    embd_rays[:].unsqueeze(1)
                    .broadcast_to([EMB_D, S_SUP, R_CORE]))
                yield (E, embd)

            def emb_first():
                """Super-tile 0 prologue: run the embedding pipeline in two
                column chunks so the PE can start layer 0 earlier."""
                sl0 = slice(0, FSUP)
                P, K = P0, K0
                E = epool.tile([EMB_X, FSUP], F32R, name="E")
                for a, b in ((0, F), (F, FSUP)):
                    Pv, Kv = P[:, a:b], K[:, a:b]
                    nc.vector.tensor_scalar(Pv, Pv, c_t[0:60, 0:1],
                                            c_t[0:60, 1:2],
                                            op0=ALU.mult, op1=ALU.add)
                    nc.vector.tensor_scalar(Kv, Pv, -1.0, -MAGIC,
                                            op0=ALU.mult, op1=ALU.add)
                    nc.vector.affine_then_add(Pv, Kv, Pv, 1.0, MAGIC)
                    nc.scalar.activation(E[0:60, a:b], Pv, AF.Sin,
                                         bias=zeros_t[0:60, 0:1], scale=TWOPI)
                nc.sync.dma_start(E[60:63, :], pts[:, sl0].bitcast(F32R))
                return E

            def sincos_dirs():
                """Per-ray [27, R_CORE] direction embedding via Cody-Waite."""
                nc.vector.tensor_scalar(pd[:], pd[:], c_t[0:24, 2:3],
                                        c_t[0:24, 3:4],
                                        op0=ALU.mult, op1=ALU.add)
                nc.vector.tensor_scalar(kd[:], pd[:], INV2PI, MAGIC,
                                        op0=ALU.mult, op1=ALU.add)
                nc.vector.tensor_scalar(kd[:], kd[:], MAGIC, None,
                                        op0=ALU.subtract)
                nc.vector.cody_waite_cascade(pd[:], pd[:], kd[:],
                                             CW1, CW2, CW3)
                nc.scalar.activation(embd_rays[0:24, :], pd[:], AF.Sin,
                                     bias=zeros_t[0:24, 0:1])
                nc.sync.dma_start(embd_rays[24:27, :], dirs[:].bitcast(F32R))

            E0 = emb_first()
            sincos_dirs()
            # st0 direction-embedding broadcast (after the embd_rays writers)
            embd0 = epool.tile([EMB_D, FSUP], F32R, name="embd")
            nc.sync.dma_start(
                embd0[:].rearrange("p (s r) -> p s r", s=S_SUP),
                embd_rays[:].unsqueeze(1)
                .broadcast_to([EMB_D, S_SUP, R_CORE]))

            emb_next = (E0, embd0)
            emb_gen = None
            for sti in range(nsup_exec):
                st = sti % NSUP
                sl = slice(st * FSUP, (st + 1) * FSUP)
                E, embd = emb_next
                emb_gen = (emb_stages((sti + 1) % NSUP)
                           if sti + 1 < nsup_exec else None)

                xa8 = apool.tile([128, 2 * FSUP], F8, name="xa8")
                xb8 = apool.tile([128, 2 * FSUP], F8, name="xb8")
                x7f = apool.tile([128, 2 * FSUP], BF16, name="x7f")
                ft8 = apool.tile([128, 2 * FSUP], F8, name="ft8")
                hf = apool.tile([128, FSUP], BF16, name="hf")
                osb = opool.tile([128, F], F32, name="osb")
                rgbsb = opool.tile([128, F], F32, name="rgbsb")

                def pair_ap(t):
                    return t[:].rearrange("p (j n) -> p j n", j=2)

                def xsl(t, chunk, a, b):
                    return t[:, chunk * FSUP + a:chunk * FSUP + b]

                def step_emb():
                    if emb_gen is None:
                        return None
                    r = next(emb_gen)
                    return r

                # ---- L0: fp32r from E -> xa8 ----
                for m in range(2):
                    for sp in range(2):
                        pt = psum.tile([128, 2 * F], F32, name="mmps",
                                       tag="mm")
                        for s2 in range(2):
                            sub = sp * 2 + s2
                            nc.tensor.matmul(
                                pt[:, s2 * F:(s2 + 1) * F],
                                w0_t[:, m * 128:(m + 1) * 128],
                                E[0:EMB_X, sub * F:(sub + 1) * F],
                                start=True, stop=True)
                        evict(pt[:], xsl(xa8, m, sp * 2 * F, (sp + 1) * 2 * F),
                              b_t[:, m:m + 1], 2 * F)

                cur = xa8
                # ---- L1..L7 ----
                for li in range(1, 8):
                    nxt = xb8 if cur is xa8 else xa8
                    if li == 7:
                        nxt = x7f
                    w3 = w8ap(w8_t[li])
                    cur3 = pair_ap(cur)
                    for sp in range(2):
                        for m in range(2):
                            pt = psum.tile([128, 2 * F], F32, name="mmps",
                                           tag="mm")
                            for s2 in range(2):
                                sub = sp * 2 + s2
                                last = li != 4
                                nc.tensor.matmul(
                                    pt[:, s2 * F:(s2 + 1) * F],
                                    w3[:, :, m * 128:(m + 1) * 128],
                                    cur3[:, :, sub * F:(sub + 1) * F],
                                    start=True, stop=last, perf_mode=DR)
                                if li == 4:
                                    nc.tensor.matmul(
                                        pt[:, s2 * F:(s2 + 1) * F],
                                        w4e_t[:, m * 128:(m + 1) * 128],
                                        E[0:EMB_X, sub * F:(sub + 1) * F],
                                        start=False, stop=True)
                            evict(pt[:],
                                  xsl(nxt, m, sp * 2 * F, (sp + 1) * 2 * F),
                                  b_t[:, 2 * li + m:2 * li + m + 1], 2 * F)
                    cur = nxt
                    if li != 4:
                        r = step_emb()
                        if r is not None:
                            emb_next = r

                # ---- feat (fp32r) -> ft8 ----
                for sp in range(2):
                    for m in range(2):
                        pt = psum.tile([128, 2 * F], F32, name="mmps",
                                       tag="mm")
                        for s2 in range(2):
                            sub = sp * 2 + s2
                            for k in range(2):
                                nc.tensor.matmul(
                                    pt[:, s2 * F:(s2 + 1) * F],
                                    wfeat_t[:, k * 256 + m * 128:
                                            k * 256 + m * 128 + 128],
                                    xsl(x7f, k, sub * F, (sub + 1) * F),
                                    start=(k == 0), stop=(k == 1))
                        evict(pt[:], xsl(ft8, m, sp * 2 * F, (sp + 1) * 2 * F),
                              b_t[:, 16 + m:17 + m], 2 * F)

                # ---- dir layer: DR(ft8) + fp32r(embd) -> hf, with the
                # den head (needs only x7f) interleaved so the eviction
                # engines aren't starved during the head-matmul phase, and
                # rgb emitted right after the hf half it needs ----
                drps = psum.tile([128, 2 * F], F32, name="drps", tag="mm")
                ft3 = pair_ap(ft8)
                for sp in range(2):
                    pt = psum.tile([128, 2 * F], F32, name="mmps", tag="mm")
                    for s2 in range(2):
                        sub = sp * 2 + s2
                        nc.tensor.matmul(
                            pt[:, s2 * F:(s2 + 1) * F],
                            w8ap(wd0x_t)[:, :, 0:128],
                            ft3[:, :, sub * F:(sub + 1) * F],
                            start=True, stop=False, perf_mode=DR)
                        nc.tensor.matmul(
                            pt[:, s2 * F:(s2 + 1) * F],
                            wd0e_t[:],
                            embd[0:EMB_D, sub * F:(sub + 1) * F],
                            start=False, stop=True)
                    for sub in (sp * 2, sp * 2 + 1):
                        for k in range(2):
                            nc.tensor.matmul(
                                drps[32 * sub:32 * sub + 32, 0:F],
                                wden_t[:, k * 32:(k + 1) * 32],
                                xsl(x7f, k, sub * F, (sub + 1) * F),
                                start=(k == 0), stop=(k == 1),
                                tile_position=(0, 32 * sub))
                    evict(pt[:], hf[:, sp * 2 * F:(sp + 1) * 2 * F],
                          b_t[:, 18:19], 2 * F)
                    for sub in (sp * 2, sp * 2 + 1):
                        nc.tensor.matmul(
                            drps[32 * sub:32 * sub + 32, F:2 * F],
                            wrgb_t[:],
                            hf[:, sub * F:(sub + 1) * F],
                            start=True, stop=True,
                            tile_position=(0, 32 * sub))

                # den eviction: rows {0,32,64,96} are the real density
                nc.vector.tensor_scalar(osb[:], drps[:, 0:F],
                                        b_t[:, 19:20], 0.0,
                                        op0=ALU.add, op1=ALU.max)
                dve_fixed(F * 1.0417 + 125.0)
                # sigmoid(z) = 0.5 + 0.5*tanh(z/2): Tanh lives in the same
                # activation table as Sin (silu_and_others), so the ACT
                # engine never swaps tables; the final affine runs on GPSIMD.
                nc.scalar.activation(rgbsb[:], drps[:, F:2 * F],
                                     AF.Tanh, bias=b_t[:, 20:21], scale=0.5)
                act_fixed(F * 0.8333 + 185.0)
                nc.gpsimd.tensor_scalar(rgbsb[:], rgbsb[:], 0.5, 0.5,
                                        op0=ALU.mult, op1=ALU.add)

                nc.sync.dma_start(
                    out[0:1, sl].rearrange("o (s n) -> (o s) n", s=NSUB),
                    osb[0:128:32, :])
                for c in range(3):
                    nc.sync.dma_start(
                        out[1 + c:2 + c, sl]
                        .rearrange("o (s n) -> (o s) n", s=NSUB),
                        rgbsb[c:128:32, :])

    _patch_act_table_loads(nc)
    nc.compile()
    _cache[key] = nc
    return nc


def _patch_act_table_loads(nc):
    """Every ACT func used here (Sin, Relu, Tanh, Identity) lives in the
    silu_and_others table, but the stock insert_act_table_loads pass picks
    tables greedily first-match (trig_and_small for Sin, exp_and_others for
    Tanh) and thrashes 2 loads x 1.3us per super-tile. Place one load of
    silu_and_others before the first activation instead (same contract:
    loads pre-placed on the Bass CFG, walrus adopts them)."""
    import types

    from concourse.hw_specs import get_activation_tables

    def _single_load(self):
        tabs = get_activation_tables(self.m.arch)
        names = list(tabs)
        idx = names.index("silu_and_others")
        allowed = tabs["silu_and_others"]
        for blk in self.main_func.blocks:
            for i, inst in enumerate(blk.instructions):
                if isinstance(inst, mybir.InstActivation):
                    assert inst.func in allowed, (
                        f"activation {inst.func} not in silu_and_others; "
                        f"single-table-load patch is invalid")
        for blk in self.main_func.blocks:
            for i, inst in enumerate(blk.instructions):
                if isinstance(inst, mybir.InstActivation):
                    load = mybir.InstLoadActFuncSet(
                        name=self.get_next_instruction_name(),
                        act_func_set_id=idx, ins=[], outs=[])
                    load.engine = mybir.EngineType.Activation
                    self.register_instruction(load)
                    blk.instructions.insert(i, load)
                    return

    nc.insert_act_table_loads = types.MethodType(_single_load, nc)


def _prep_inputs(inputs):
    """Host-side shard + transpose prep. Returns list of per-core dicts."""
    f32 = np.float32
    sp = np.ascontiguousarray(inputs["sample_points"], dtype=f32)
    dirs = np.ascontiguousarray(inputs["directions"], dtype=f32)

    dirs_all = dirs.T.copy()                          # [3, 4096]

    def wt(w):  # [out, in] -> [in, out]
        return np.ascontiguousarray(w.T, dtype=f32)

    def kpack(w, m):  # [m, 2*128k] -> [128, 2, m] -> [128, 2*m]
        t = wt(w)                                     # [in, m]
        kin = t.shape[0] // 2
        assert kin == 128
        return np.ascontiguousarray(
            t.reshape(2, 128, m).transpose(1, 0, 2).reshape(128, 2 * m))

    # P-broadcast row order: r = c*20 + half*10 + h  ->  emb idx half*30+c*10+h
    permx = np.arange(63)
    for r in range(60):
        c, half, h = r // 20, (r % 20) // 10, r % 10
        permx[r] = half * 30 + c * 10 + h
    permd = np.arange(27)
    for r in range(24):
        c, half, h = r // 8, (r % 8) // 4, r % 4
        permd[r] = half * 12 + c * 4 + h

    w8cat = []
    shared = {}
    shared["w0"] = wt(inputs["Wx0"])[permx]           # [63, 256]
    for i in range(1, 8):
        w = inputs[f"Wx{i}"]
        if i == 4:
            w8cat.append(kpack(w[:, :256], 256).astype(NP_F8))
            shared["w4e"] = wt(w[:, 256:])[permx]     # [63, 256]
        else:
            w8cat.append(kpack(w, 256).astype(NP_F8))
    shared["w8mid"] = np.ascontiguousarray(np.concatenate(w8cat, axis=1))
    shared["wfeat"] = kpack(inputs["Wfeat"], 256).astype(NP_BF16)
    wden_pad = np.zeros((32, 256), dtype=f32)
    wden_pad[0] = inputs["Wden"][0]
    shared["wden"] = kpack(wden_pad, 32).astype(NP_BF16)  # [128, 64]
    wd0 = inputs["Wd0"]                               # [128, 283]
    shared["wd0x"] = kpack(wd0[:, :256], 128).astype(NP_F8)
    shared["wd0e"] = np.ascontiguousarray(wt(wd0)[256:][permd])  # [27, 128]
    wrgb_pad = np.zeros((32, 128), dtype=f32)
    wrgb_pad[0:3] = inputs["Wrgb"]
    shared["wrgb"] = wt(wrgb_pad).astype(NP_BF16)     # [128, 32]

    bias = np.zeros((128, 21), dtype=f32)
    for li in range(8):
        b = inputs[f"bx{li}"]
        bias[:, 2 * li] = b[:128]
        bias[:, 2 * li + 1] = b[128:]
    bias[:, 16] = inputs["bfeat"][:128]
    bias[:, 17] = inputs["bfeat"][128:]
    bias[:, 18] = inputs["bd0"]
    for s in range(4):
        bias[32 * s, 19] = inputs["bden"][0]
        # rgb eviction computes tanh(psum*0.5 + brgb*0.5)
        bias[32 * s:32 * s + 3, 20] = inputs["brgb"] * 0.5

    consts = np.zeros((128, 4), dtype=f32)
    consts[0:60, 0] = 2.0 ** (np.arange(60) % 10) / (2 * np.pi)
    consts[0:60, 1] = np.where((np.arange(60) % 20) // 10 == 1, 0.25, 0.0)
    consts[0:24, 2] = 2.0 ** (np.arange(24) % 4)
    consts[0:24, 3] = np.where((np.arange(24) % 8) // 4 == 1, np.pi / 2, 0.0)

    in_maps = []
    for c in range(N_CORES):
        m = dict(shared)
        # sample-major: [3, S, R] flattened to [3, NPTS]
        blk = sp[c * R_CORE:(c + 1) * R_CORE]         # [R, S, 3]
        m["pts"] = np.ascontiguousarray(
            blk.transpose(2, 1, 0).reshape(3, NPTS))
        m["dirs"] = np.ascontiguousarray(
            dirs_all[:, c * R_CORE:(c + 1) * R_CORE])
        m["biases"] = bias
        m["consts"] = consts
        in_maps.append(m)
    return in_maps


def kernel(**inputs) -> np.ndarray:
    nc = _build()
    in_maps = _prep_inputs(inputs)
    res = run_bass_kernel_spmd(nc, in_maps, core_ids=list(range(N_CORES)))
    outs = []
    for c in range(N_CORES):
        o = res.results[c]["out"]                     # [4, NPTS] sample-major
        outs.append(o.reshape(4, S, R_CORE).transpose(2, 1, 0))
    return np.concatenate(outs, axis=0)
